# revision 54
# baseline (speedup 1.0000x reference)
"""Trainium2 Bass kernel: gumbel-softmax-argmax embedding lookup (end-to-end).

Reference math (nn_End2End_49495203119139):
    hot  = argmax_V(softmax((logits + gumbel)/tau))       == argmax_V(logits+gumbel)
    row  = grid_sample-nearest index map of hot            == ROWMAP[hot]  (LUT)
    tok_emb = W[row][:, col_map]
    inputs_embeds = tok_emb * mask
    psg_roll = roll(psg_ids, 1, axis=1); psg_roll[:,0] = 1
    extr  = (1 - mask[:, ::-1]) * psg_roll
    trunc = rotate_right(extr, shifts) with shifts = mask.sum(-1)   (per row)
    flag  = cumsum(trunc != 0, -1) > 0
    out   = inputs_embeds + where(flag, W[trunc], 0)

Sharding: data-parallel over batch. B=16 over 8 cores -> 2 batch rows
(= 2 token tiles of 128) per core; embedding tables replicated.

Host precomputes (cheap, O(B*L) index arithmetic + one-time table reshuffles):
  - W2Z [V+1,E] = W[ROWMAP][:, COLMAP] with a zero row appended at index V
  - WZ  [V+1,E] = W with a zero row appended at index V
  - psg_idx [B,L] = flag ? trunc : V     (zero-row redirect replaces `where`)
  - mask_f  [B,L], vinv = (1-mask)*V     (token index redirect coefficients)

Per-core device plan (memory regime: streaming logits+gumbel, 66 MB/core,
~183 us HBM floor at 360 GB/s per core; sim/HW 204.0 us vs 212.4 baseline):
  - stream the vocab in chunks per 128-token tile (bulk 2048 wide, with a
    gradually tapered chunk schedule at each tile's end): HWDGE-load the
    logits chunk (ACT/SP queues alternate), add the gumbel chunk in the DMA
    datapath (SWDGE CCE inline add, <=2048 elements per descriptor), then
    DVE Max + MaxIndex written in place into flat [128, 8*nchunks] stats
    tiles. The taper keeps the DVE pipeline (which runs ~1 chunk + 900 ns
    sem behind the bus) nearly drained when the final accumulate lands.
  - tile 0's taper is interleaved 1:1 with tile 1's early bulk chunks so
    its latency-bound round trips hide inside tile 1's streaming; separate
    per-tile load-buffer pools keep tile 1's loads independent of tile 0's
    DVE progress.
  - tail per tile, split in two: the winner over all but the last 3 chunks
    (Max+MaxIndex over the flat stats + iota-select of idx+base) runs while
    those chunks' accumulates are still in flight; the merge then folds the
    last 3 chunks with strict-> compares (argmax first-occurrence ties).
    tok_idx = hot*mask + (1-mask)*V; one indirect gather of W2Z[tok_idx]
    with compute_op=add accumulates the token embedding onto the
    psg embedding (gathered from WZ[psg_idx]) in SBUF; Pool-queue store.
  - the psg gathers are pinned LATE via an artificial data dependency on a
    mid-stream chunk's stats (the scheduler would otherwise hoist them to
    t=0); they are the bus filler for tile 1's taper stretch, where the
    load->accum->max round trips otherwise leave the bus idle.
"""

import numpy as np

B = 16
L = 128
V = 32128
E = 768
N_CORES = 8
B_LOC = B // N_CORES          # batch rows (= 128-token tiles) per core
BULK = 2048                   # bulk vocab chunk width (<=2048: one CCE-add
                              # descriptor per partition row)
# long gradual taper: the DVE pipeline runs ~1 chunk + 900ns behind the DMA
# stream, so chunk sizes must shrink smoothly toward the end for the final
# max/max_index work to be tiny when the last accumulate lands
TAPER = (2048, 1536, 1280, 1024, 768, 512, 256, 128)
NEG = -3.0e38


def _spans(d):
    bulk = d.get("BULK", BULK)
    taper = list(d.get("TAPER", TAPER))
    t_sum = sum(taper)
    assert (V - t_sum) % bulk == 0, (V, t_sum, bulk)
    spans = [(c * bulk, bulk) for c in range((V - t_sum) // bulk)]
    lo = V - t_sum
    for s in taper:
        spans.append((lo, s))
        lo += s
    assert lo == V
    return spans


def _slots(d):
    """Stats slots: each load span splits into <=SPLIT-wide sub-slots so the
    DVE max/max_index pipeline quantum stays small. Returns (spans, slots,
    chunk_slots) where chunk_slots[c] lists slot indices of load chunk c."""
    spans = _spans(d)
    split = d.get("SPLIT", 0)
    slots = []
    chunk_slots = []
    for lo, csz in spans:
        ids = []
        off = 0
        while off < csz:
            sz = min(split, csz - off) if split else csz
            ids.append(len(slots))
            slots.append((lo + off, sz))
            off += sz
        chunk_slots.append(ids)
    return spans, slots, chunk_slots


def _build(nc_mod, dims=None):
    import concourse.tile as tile
    from concourse import mybir
    from concourse.bass import IndirectOffsetOnAxis

    d = dims or {}
    spans, slots, chunk_slots = _slots(d)
    nsp = len(spans)
    n_slots = len(slots)
    F = 8 * n_slots
    b_loc = d.get("B_LOC", B_LOC)
    rows = b_loc * L
    lbufs = d.get("LBUFS", 6)
    bulk = d.get("BULK", BULK)
    # Pool-queue positions (within the NEXT tile's accumulate stream) at which
    # the previous tile's tail gather / store are interleaved
    k_gather = d.get("K_GATHER", 2)
    k_store = d.get("K_STORE", 4)
    store_eng = d.get("STORE_ENG", "pool")

    nc = nc_mod
    f32 = mybir.dt.float32
    i32 = mybir.dt.int32
    u32 = mybir.dt.uint32
    Op = mybir.AluOpType
    AX = mybir.AxisListType

    logits_h = nc.dram_tensor("logits", [rows, V], f32, kind="ExternalInput")
    gumbel_h = nc.dram_tensor("gumbel", [rows, V], f32, kind="ExternalInput")
    pix_h = nc.dram_tensor("pix", [rows, 1], i32, kind="ExternalInput")
    mv_h = nc.dram_tensor("mv", [rows, 2], f32, kind="ExternalInput")
    aux_h = nc.dram_tensor("aux", [L, 2 * F], f32, kind="ExternalInput")
    w2z_h = nc.dram_tensor("w2z", [V + 1, E], f32, kind="ExternalInput")
    wz_h = nc.dram_tensor("wz", [V + 1, E], f32, kind="ExternalInput")
    out_h = nc.dram_tensor("out", [rows, E], f32, kind="ExternalOutput")

    with tile.TileContext(nc) as tc:
        with (
            tc.tile_pool(name="lpool0", bufs=d.get("LBUFS0", lbufs)) as lpool0,
            tc.tile_pool(name="lpool1", bufs=d.get("LBUFS1", lbufs)) as lpool1,
            tc.tile_pool(name="misc", bufs=2) as misc,
        ):
            lpools = [lpool0, lpool1]
            stats = small = emb = consts = misc
            # ---- tiny constant loads (ACT queue, ahead of its odd-chunk
            # loads; they land on the bus before the first big transfer) ----
            aux_sb = consts.tile([L, 2 * F], f32)
            nc.scalar.dma_start(out=aux_sb[:], in_=aux_h[:])
            iota_f = aux_sb[:, 0:F]
            bases_f = aux_sb[:, F:2 * F]
            pix_sb = []
            mv_sb = []
            for t in range(b_loc):
                tok = slice(t * L, (t + 1) * L)
                p = consts.tile([L, 1], i32, tag=f"pix{t}")
                nc.scalar.dma_start(out=p[:], in_=pix_h[tok, :])
                pix_sb.append(p)
                m = consts.tile([L, 2], f32, tag=f"mv{t}")
                nc.scalar.dma_start(out=m[:], in_=mv_h[tok, :])
                mv_sb.append(m)

            # out tiles; psg gathers are deferred into tile 1's taper stretch
            # as bus filler (see schedule below)
            outts = []
            for t in range(b_loc):
                outt = emb.tile([L, E], f32, tag="outt")
                outts.append(outt)

            # The psg gathers must execute LATE (they are the bus filler for
            # tile 1's taper stretch, where the pipeline otherwise idles).
            # A copy of the index tile that depends on a chosen stream chunk's
            # m_flat slot pins each gather after that chunk — the scheduler
            # cannot hoist it.
            pix2 = []
            for t in range(b_loc):
                p2 = consts.tile([L, 1], i32, tag=f"pix2_{t}")
                pix2.append(p2)

            def psg_dep(t, anchor_chunk):
                a = 8 * chunk_slots[anchor_chunk][-1]
                zm = small.tile([L, 1], f32, tag=f"zm{t}")
                nc.vector.tensor_scalar(
                    zm[:], m_flat[1][:, a:a + 1], 0.0, None, op0=Op.mult)
                zi = small.tile([L, 1], i32, tag=f"zi{t}")
                nc.vector.tensor_copy(out=zi[:], in_=zm[:])
                nc.vector.tensor_tensor(
                    out=pix2[t][:], in0=pix_sb[t][:], in1=zi[:], op=Op.add)

            def psg_gather(t):
                nc.gpsimd.indirect_dma_start(
                    out=outts[t][:], out_offset=None, in_=wz_h[:],
                    in_offset=IndirectOffsetOnAxis(ap=pix2[t][:, 0:1], axis=0),
                )

            def issue_chunk(t, c, lo, csz, pend):
                """Issue one chunk's load+accum+max+max_index; returns nothing.
                pend: list collecting deferred Pool-queue callbacks."""
                tok = slice(t * L, (t + 1) * L)
                if c >= nsp - d.get("LAST_FREE", 0):
                    # dedicated tiles: the final chunks' loads/accums are not
                    # buffer-gated, removing their round-trip latency from the
                    # end-of-stream critical chain
                    lt = misc.tile([L, csz], f32, tag=f"ltz{t}_{c}")
                else:
                    lt = lpools[t].tile([L, bulk], f32, tag="lt")
                ldeng = nc.scalar if c % 2 else nc.sync
                ldeng.dma_start(out=lt[:, 0:csz], in_=logits_h[tok, lo:lo + csz])
                for sid in chunk_slots[c]:
                    lo_s, sz_s = slots[sid]
                    sl = slice(lo_s - lo, lo_s - lo + sz_s)
                    nc.gpsimd.dma_start(
                        out=lt[:, sl], in_=gumbel_h[tok, lo_s:lo_s + sz_s],
                        accum_op=Op.add)
                    s = slice(8 * sid, 8 * sid + 8)
                    nc.vector.max(out=m_flat[t][:, s], in_=lt[:, sl])
                    nc.vector.max_index(
                        out=i_flat[t][:, s], in_max=m_flat[t][:, s],
                        in_values=lt[:, sl])

            n_last = d.get("N_LAST", 3)  # stat slots folded serially in merge

            def tail_partial(t):
                """Winner among slots 0..n_slots-1-n_last: runs while the last
                (small) chunks' accumulates are still in flight."""
                Fp = 8 * (n_slots - n_last)
                # idx+base in f32 for the partial range (hidden behind the
                # last chunks' DMA flight)
                ibpa = small.tile([L, F], f32, tag="ibpa")
                nc.vector.tensor_copy(out=ibpa[:, 0:Fp], in_=i_flat[t][:, 0:Fp])
                nc.vector.tensor_tensor(
                    out=ibpa[:, 0:Fp], in0=ibpa[:, 0:Fp], in1=bases_f[:, 0:Fp],
                    op=Op.add)
                mm8a = small.tile([L, 8], f32, tag="mm8a")
                nc.vector.max(out=mm8a[:], in_=m_flat[t][:, 0:Fp])
                jj8a = small.tile([L, 8], u32, tag="jj8a")
                nc.vector.max_index(
                    out=jj8a[:], in_max=mm8a[:], in_values=m_flat[t][:, 0:Fp])
                jfa = small.tile([L, 1], f32, tag="jfa")
                nc.vector.tensor_copy(out=jfa[:], in_=jj8a[:, 0:1])
                sela = small.tile([L, F], f32, tag="sela")
                nc.vector.scalar_tensor_tensor(
                    out=sela[:, 0:Fp], in0=iota_f[:, 0:Fp], scalar=jfa[:, 0:1],
                    in1=ibpa[:, 0:Fp], op0=Op.is_equal, op1=Op.mult)
                hota = small.tile([L, 1], f32, tag="hota")
                nc.vector.reduce_max(out=hota[:], in_=sela[:, 0:Fp], axis=AX.X)
                return mm8a, hota

            def tail_merge(t, mm8a, hota):
                """Fold the last n_last chunks' max/argmax into the partial
                winner, one at a time in vocab order. Strict > keeps argmax
                first-occurrence tie semantics."""
                gcur, hcur = mm8a[:, 0:1], hota[:]
                for k in range(n_slots - n_last, n_slots):
                    sL = slice(8 * k, 8 * k + 1)
                    bet = small.tile([L, 1], f32, tag=f"bet{k}")
                    nc.vector.tensor_tensor(
                        out=bet[:], in0=m_flat[t][:, sL], in1=gcur, op=Op.is_gt)
                    ib1 = small.tile([L, 1], f32, tag=f"ib1_{k}")
                    nc.vector.tensor_scalar(
                        ib1[:], i_flat[t][:, sL], float(slots[k][0]), None,
                        op0=Op.add)
                    d1 = small.tile([L, 1], f32, tag=f"d1_{k}")
                    nc.vector.tensor_tensor(
                        out=d1[:], in0=ib1[:], in1=hcur, op=Op.subtract)
                    hnew = small.tile([L, 1], f32, tag=f"hnew{k}")
                    # hot = bet*(ibp_k - hot) + hot
                    nc.vector.scalar_tensor_tensor(
                        out=hnew[:], in0=bet[:], scalar=d1[:, 0:1], in1=hcur,
                        op0=Op.mult, op1=Op.add)
                    hcur = hnew[:]
                    if k < n_slots - 1:
                        gnew = small.tile([L, 1], f32, tag=f"gnew{k}")
                        nc.vector.tensor_tensor(
                            out=gnew[:], in0=m_flat[t][:, sL], in1=gcur, op=Op.max)
                        gcur = gnew[:]
                tokf = small.tile([L, 1], f32, tag="tokf")
                # tok_idx = hot*mask + (1-mask)*V
                nc.vector.tensor_scalar(
                    tokf[:], hcur, mv_sb[t][:, 0:1], None, op0=Op.mult)
                nc.vector.tensor_tensor(
                    out=tokf[:], in0=tokf[:], in1=mv_sb[t][:, 1:2], op=Op.add)
                toki = small.tile([L, 1], i32, tag="toki")
                nc.vector.tensor_copy(out=toki[:], in_=tokf[:])
                return toki

            tail_split = d.get("TAIL_SPLIT", 1)

            def tail_gather(t, toki):
                if tail_split == 2:
                    h = E // 2
                    nc.gpsimd.indirect_dma_start(
                        out=outts[t][:, 0:h], out_offset=None, in_=w2z_h[:],
                        in_offset=IndirectOffsetOnAxis(ap=toki[:, 0:1], axis=0),
                        compute_op=Op.add,
                    )
                    nc.gpsimd.indirect_dma_start(
                        out=outts[t][:, h:E], out_offset=None, in_=w2z_h[:],
                        in_offset=IndirectOffsetOnAxis(ap=toki[:, 0:1], axis=0),
                        element_offset=h, compute_op=Op.add,
                    )
                else:
                    nc.gpsimd.indirect_dma_start(
                        out=outts[t][:], out_offset=None, in_=w2z_h[:],
                        in_offset=IndirectOffsetOnAxis(ap=toki[:, 0:1], axis=0),
                        compute_op=Op.add,
                    )

            def tail_store(t):
                tok = slice(t * L, (t + 1) * L)
                if tail_split == 2:
                    h = E // 2
                    nc.sync.dma_start(out=out_h[tok, 0:h], in_=outts[t][:, 0:h])
                    nc.scalar.dma_start(out=out_h[tok, h:E], in_=outts[t][:, h:E])
                    return
                eng = {"pool": nc.gpsimd, "sp": nc.sync, "act": nc.scalar,
                       "dve": nc.vector}[store_eng]
                eng.dma_start(out=out_h[tok, :], in_=outts[t][:])

            m_flat = [None] * b_loc
            i_flat = [None] * b_loc
            for t in range(b_loc):
                mft = stats.tile([L, F], f32, tag="m_flat")
                ift = stats.tile([L, F], u32, tag="i_flat")
                m_flat[t], i_flat[t] = mft, ift

            # Global issue schedule: t0 bulk; then t0's taper interleaved 1:1
            # with t1's early bulk (so t0's latency-bound taper round trips
            # hide inside t1's streaming instead of blocking the in-order
            # queues); then the rest of t1. Tail compute is issued right after
            # the owning tile's chunks; t0's gather/store are placed a few
            # chunks into t1's remaining stream (their waits are met by then).
            n_taper = len(d.get("TAPER", TAPER))
            nb = nsp - n_taper
            part_slot = n_slots - 1 - n_last
            part_chunk = next(c for c in range(nsp)
                              if part_slot in chunk_slots[c])
            mode = d.get("SCHED", "old")
            if mode == "dualtaper":
                # Both tiles' tapers interleaved 1:1, tile 0 offset 'lead'
                # chunks earlier so its tail (merge/gather/store) fills the
                # end-region bus bubbles without contending with tile 1's.
                lead = d.get("LEAD", 2)
                K = nsp - n_taper - lead
                order = [(0, c) for c in range(nb)]
                order += [(1, c) for c in range(K)]
                for i in range(n_taper):
                    order.append((0, nb + i))
                    order.append((1, K + i))
                order += [(1, c) for c in range(K + n_taper, nsp)]
                a0 = d.get("ANCHOR0", max(0, K - 4))
                a1 = d.get("ANCHOR1", min(nsp - 3, K + 3))
                sched = []
                for t, c in order:
                    sched.append(("chunk", t, c))
                    if c == part_chunk:
                        sched.append(("partial", t, 0))
                    if t == 0 and c == nsp - 1:
                        sched += [("merge", 0, 0), ("gather", 0, 0),
                                  ("store", 0, 0)]
                    if t == 1 and c == 0:
                        sched += [("psgdep", 0, a0), ("psg", 0, 0),
                                  ("psgdep", 1, a1), ("psg", 1, 0)]
                sched += [("merge", 1, 0), ("gather", 1, 0), ("store", 1, 0)]
            else:
                sched = [("chunk", 0, c) for c in range(nb)]
                for i in range(n_taper):
                    if i < nsp:
                        sched.append(("chunk", 1, i))
                    sched.append(("chunk", 0, nb + i))
                    if nb + i == part_chunk:
                        sched.append(("partial", 0, 0))
                    if nb + i == nsp - 1:
                        sched.append(("merge", 0, 0))
                a0 = d.get("ANCHOR0", 6)
                a1 = d.get("ANCHOR1", 13)
                for j, c in enumerate(range(n_taper, nsp)):
                    if j == 0:
                        sched.append(("psgdep", 0, a0))
                        sched.append(("psg", 0, 0))
                        sched.append(("psgdep", 1, a1))
                        sched.append(("psg", 1, 0))
                    if j == k_gather:
                        sched.append(("gather", 0, 0))
                    if j == k_store:
                        sched.append(("store", 0, 0))
                    sched.append(("chunk", 1, c))
                    if c == part_chunk:
                        sched.append(("partial", 1, 0))
                sched += [("merge", 1, 0), ("gather", 1, 0), ("store", 1, 0)]

            part = [None] * b_loc
            toki = [None] * b_loc
            for kind, t, c in sched:
                if kind == "chunk":
                    lo, csz = spans[c]
                    issue_chunk(t, c, lo, csz, None)
                elif kind == "partial":
                    part[t] = tail_partial(t)
                elif kind == "merge":
                    toki[t] = tail_merge(t, *part[t])
                elif kind == "psgdep":
                    psg_dep(t, c)
                elif kind == "psg":
                    psg_gather(t)
                elif kind == "gather":
                    tail_gather(t, toki[t])
                elif kind == "store":
                    tail_store(t)

    return nc


_BUILD_CACHE = {}


def _get_module(dims_key=None, dims=None):
    key = dims_key
    if key not in _BUILD_CACHE:
        import concourse.bacc as bacc

        nc = bacc.Bacc("TRN2", target_bir_lowering=False, debug=False)
        _build(nc, dims)
        nc.compile()
        _BUILD_CACHE[key] = nc
    return _BUILD_CACHE[key]


_MAPS_CACHE = {}


def _nearest_maps():
    """Replicate the reference's f32 grid_sample-nearest index maps with jnp
    on the same backend the reference runs on (bit-exact by construction)."""
    if "maps" not in _MAPS_CACHE:
        import jax.numpy as jnp

        def nearest(size):
            lin = jnp.linspace(-1.0, 1.0, size)
            ix = ((lin + 1.0) * size - 1.0) / 2.0
            return np.asarray(jnp.clip(jnp.round(ix), 0, size - 1).astype(jnp.int32))

        _MAPS_CACHE["maps"] = (nearest(V), nearest(E))
    return _MAPS_CACHE["maps"]


def _aux_array(dims=None):
    _, slots, _ = _slots(dims or {})
    F = 8 * len(slots)
    iota = np.arange(F, dtype=np.float32)
    bases = np.repeat(np.array([lo for lo, _ in slots], dtype=np.float32), 8)
    row = np.concatenate([iota, bases])
    return np.ascontiguousarray(np.broadcast_to(row, (L, 2 * F)).astype(np.float32))


# test/dev hooks: set TRACE=True before calling kernel() to capture an NTFF
# profile; the BassKernelResults of the last run is stored in LAST_RESULT.
TRACE = False
LAST_RESULT = None


def kernel(logits, rwrt_attention_mask, psg_input_ids, word_embeddings, gumbel_noise):
    from concourse.bass_utils import run_bass_kernel_spmd

    logits = np.ascontiguousarray(np.asarray(logits, dtype=np.float32))
    gumbel = np.ascontiguousarray(np.asarray(gumbel_noise, dtype=np.float32))
    mask = np.asarray(rwrt_attention_mask, dtype=np.int32)
    psg = np.asarray(psg_input_ids, dtype=np.int32)
    wte = np.asarray(word_embeddings, dtype=np.float32)

    rowmap, colmap = _nearest_maps()
    zrow = np.zeros((1, E), dtype=np.float32)
    w2z = np.ascontiguousarray(np.vstack([wte[rowmap][:, colmap], zrow]))
    wz = np.ascontiguousarray(np.vstack([wte, zrow]))

    # passage branch index arithmetic (exact integer ops, O(B*L))
    psg_roll = np.roll(psg, 1, axis=1)
    psg_roll[:, 0] = 1
    flipped = 1 - mask[:, ::-1]
    extr = flipped * psg_roll
    shifts = mask.sum(axis=1)
    pos = (np.arange(L)[None, :] - shifts[:, None]) % L
    trunc = np.take_along_axis(extr, pos, axis=1)
    flag = np.cumsum(trunc != 0, axis=1) > 0
    pix = np.where(flag, trunc, V).astype(np.int32)

    maskf = mask.astype(np.float32)
    vinv = (1.0 - maskf) * float(V)
    mv = np.ascontiguousarray(
        np.stack([maskf, vinv], axis=-1).astype(np.float32))  # [B, L, 2]
    aux = _aux_array()

    nc = _get_module()

    in_maps = []
    for m in range(N_CORES):
        sl = slice(m * B_LOC, (m + 1) * B_LOC)
        in_maps.append({
            "logits": logits[sl].reshape(B_LOC * L, V),
            "gumbel": gumbel[sl].reshape(B_LOC * L, V),
            "pix": np.ascontiguousarray(pix[sl].reshape(B_LOC * L, 1)),
            "mv": np.ascontiguousarray(mv[sl].reshape(B_LOC * L, 2)),
            "aux": aux,
            "w2z": w2z,
            "wz": wz,
        })

    global LAST_RESULT
    try:
        LAST_RESULT = run_bass_kernel_spmd(nc, in_maps, list(range(N_CORES)), trace=TRACE)
    except Exception:
        # the axon-relayed device occasionally reports a transient
        # NRT_EXEC_UNIT_UNRECOVERABLE on the first execution after long
        # sessions; a straight re-run recovers it
        import time as _time

        _time.sleep(2.0)
        LAST_RESULT = run_bass_kernel_spmd(nc, in_maps, list(range(N_CORES)), trace=TRACE)
    res = LAST_RESULT.results
    out = np.concatenate(
        [res[m]["out"].reshape(B_LOC, L, E) for m in range(N_CORES)], axis=0
    )
    return out
